# revision 12
# baseline (speedup 1.0000x reference)
"""GCN (7-layer) + mean-pool Trainium2 kernel, 8-core SPMD — v2.

vs v1 baseline:
- One-hot segment-sum matrices S (GCN norm folded in) built on the HOST and
  streamed from DRAM as bf16 — no IS_EQ on Vector, no dinv tensors.
- Self-loop (diagonal) term handled by one matmul per window against the
  node-major shard (S block = diag(dinv^2)) — removes 6% of gather indices.
- Dst windows are 128 wide; within each (superwindow, src-block) chunk the
  windows' messages are packed contiguously (common per-window segment
  lengths = max count over cores), so gather index count is ~message count
  instead of per-cell 128-tile rounding.
- The per-layer feature table is AllGathered directly into 4 zero-offset
  block tensors (no table->tblk copy).
"""
import sys
import types

import numpy as np
import ml_dtypes

import concourse.bacc as bacc
import concourse.mybir as mybir
import concourse.tile as tile
from concourse.bass_utils import run_bass_kernel_spmd
from concourse.masks import make_identity

# ---------------- problem constants (hardcoded per spec) ----------------
N = 100000
E = 1600000
F = 128
L = 7
G = 512
NC = 8
NLOC = N // NC              # 12500
NPAD = 12800
WIN = 128
NWIN = NPAD // WIN          # 100
SWW = 12                    # windows per superwindow (3 PSUM banks)
NSW = (NWIN + SWW - 1) // SWW   # 9
NBLK = 4
BLKLOC = NPAD // NBLK       # 3200
NBLKROWS = BLKLOC * NC      # 25600 (int16-safe)
NBLOCK128 = NPAD // 128     # 100
GCH = 24                    # gather call size (tiles) — ring-capacity bound

F32 = mybir.dt.float32
BF16 = mybir.dt.bfloat16
I16 = mybir.dt.int16

_SO_PATH = "/opt/axon/libaxon_pjrt.so"


def _install_profile_hook():
    if "antenv.axon_hooks" in sys.modules:
        return
    try:
        from trn_agent_boot.trn_boot import _ntff_profile_via_ctypes
    except Exception:
        return
    hook = _ntff_profile_via_ctypes(_SO_PATH)
    mod = types.ModuleType("antenv.axon_hooks")
    mod.get_axon_ntff_profile_hook = lambda: hook
    mod.set_axon_ntff_profile_hook = lambda h: None
    sys.modules["antenv.axon_hooks"] = mod
    try:
        import antenv

        antenv.axon_hooks = mod
    except Exception:
        pass


def _wrap_idx(idx):
    """[n] int (n % 128 == 0) -> dma_gather wrapped layout [128, n//16] int16
    (replicated into each 16-partition group)."""
    n = idx.shape[0]
    blk = idx.reshape(n // 16, 16).T.astype(np.int16)
    return np.ascontiguousarray(np.tile(blk, (8, 1)))


def _sw_windows(sw):
    w0 = sw * SWW
    return w0, min(NWIN, w0 + SWW) - w0


def _structure(cnt_max):
    """Compile-time layout from per-cell max counts [NWIN, NBLK].

    Returns per-(sw,b) chunk info and the S-block (matmul) schedule.
    chunks[(sw,b)] = dict(tbase, tiles, seg_start (per local window),
                          pairs=[(t, wl, gid)...] t-major)
    diag_gid[sw] = first gid of the sw's diag blocks (SWW contiguous)
    last_of_window[w] = (b, t) of the final matmul in w's accumulation chain
                        or None if no edge blocks at all
    """
    chunks = {}
    diag_gid = np.zeros(NSW, np.int64)
    gid = 0
    tbase = 0
    for sw in range(NSW):
        w0, nw = _sw_windows(sw)
        diag_gid[sw] = gid
        gid += nw
        for b in range(NBLK):
            seg = cnt_max[w0 : w0 + nw, b]
            starts = np.concatenate([[0], np.cumsum(seg)])
            total = int(starts[-1])
            tiles = -(-total // 128)
            pairs = []
            for t in range(tiles):
                lo, hi = t * 128, (t + 1) * 128
                for wl in range(nw):
                    if starts[wl] < hi and starts[wl + 1] > lo:
                        pairs.append((t, wl, gid))
                        gid += 1
            chunks[(sw, b)] = dict(
                tbase=tbase, tiles=tiles, starts=starts, pairs=pairs
            )
            tbase += tiles
    # last matmul of each window's chain
    last_of_window = {}
    for sw in range(NSW):
        w0, nw = _sw_windows(sw)
        for b in range(NBLK):
            for t, wl, g in chunks[(sw, b)]["pairs"]:
                last_of_window[w0 + wl] = (b, t, g)
    return chunks, diag_gid, last_of_window, gid, tbase


def _preprocess(edge_index):
    src = np.asarray(edge_index[0], dtype=np.int64)
    dst = np.asarray(edge_index[1], dtype=np.int64)
    deg = (np.bincount(dst, minlength=N) + 1).astype(np.float64)  # + self loop
    dinv = 1.0 / np.sqrt(deg)
    val_all = (dinv[src] * dinv[dst]).astype(np.float32)

    s_core = src // NLOC
    s_loc = src % NLOC
    s_blk = s_loc // BLKLOC
    s_row = s_core * BLKLOC + (s_loc % BLKLOC)

    d_core = dst // NLOC
    d_loc = dst % NLOC
    d_win = d_loc // WIN
    d_off = d_loc % WIN

    counts = np.zeros((NC, NWIN, NBLK), np.int64)
    for c in range(NC):
        m = d_core == c
        np.add.at(counts[c], (d_win[m], s_blk[m]), 1)
    cnt_max = counts.max(axis=0)  # [NWIN, NBLK]

    chunks, diag_gid, last_of_window, nblocks, tot_tiles = _structure(cnt_max)

    idxw_all = np.zeros((NC, 128, tot_tiles * 8), np.int16)
    s_all = np.zeros((NC, 128, nblocks * 128), ml_dtypes.bfloat16)
    for c in range(NC):
        m = d_core == c
        sr = s_row[m]
        sb = s_blk[m]
        wv = d_win[m]
        dofs = d_off[m]
        vals = val_all[m]
        swv = wv // SWW

        idx_flat = np.zeros(tot_tiles * 128, np.int64)
        s_dense = np.zeros((nblocks, 128, WIN), np.float32)

        # per (sw,b): flat positions = chunk_start*128 + seg_start[wl] + rank
        ordm = np.lexsort((wv, sb, swv))
        sr, sb, wv, dofs, vals = sr[ordm], sb[ordm], wv[ordm], dofs[ordm], vals[ordm]
        swv = wv // SWW
        cell_id = wv * NBLK + sb
        change = np.empty(len(cell_id), bool)
        if len(cell_id):
            change[0] = True
            change[1:] = cell_id[1:] != cell_id[:-1]
        run_start = np.maximum.accumulate(
            np.where(change, np.arange(len(cell_id)), 0)
        )
        rank = np.arange(len(cell_id)) - run_start

        # flat index (within the whole idx stream) of each message
        seg_start = np.zeros((NWIN, NBLK), np.int64)
        chunk_t0 = np.zeros((NWIN, NBLK), np.int64)
        for (sw, b), ch in chunks.items():
            w0, nw = _sw_windows(sw)
            seg_start[w0 : w0 + nw, b] = ch["starts"][:-1]
            chunk_t0[w0 : w0 + nw, b] = ch["tbase"]
        q = seg_start[wv, sb] + rank          # position within (sw,b) stream
        flat = chunk_t0[wv, sb] * 128 + q
        idx_flat[flat] = sr
        idxw_all[c] = _wrap_idx(idx_flat)

        # S block id for each message: pairs of its (sw,b) chunk
        t_of = q // 128
        gid_msg = np.full(len(q), -1, np.int64)
        for (sw, b), ch in chunks.items():
            w0, nw = _sw_windows(sw)
            if not ch["pairs"]:
                continue
            g2 = np.full((ch["tiles"], nw), -1, np.int64)
            for t, wl, g in ch["pairs"]:
                g2[t, wl] = g
            sel = (swv == sw) & (sb == b)
            if not sel.any():
                continue
            gid_msg[sel] = g2[t_of[sel], wv[sel] - w0]
        assert (gid_msg >= 0).all()
        s_dense[gid_msg, q % 128, dofs] = vals

        # diag blocks: window w, node p -> dinv^2 at [p, p]
        dloc_pad = np.arange(NPAD)
        valid = dloc_pad < NLOC
        d2 = np.zeros(NPAD, np.float32)
        d2[:NLOC] = (dinv[c * NLOC : (c + 1) * NLOC] ** 2).astype(np.float32)
        for sw in range(NSW):
            w0, nw = _sw_windows(sw)
            for wl in range(nw):
                g = int(diag_gid[sw]) + wl
                p = np.arange(128)
                s_dense[g, p, p] = d2[(w0 + wl) * 128 + p]
        s_all[c] = (
            s_dense.transpose(1, 0, 2).reshape(128, nblocks * 128)
        ).astype(ml_dtypes.bfloat16)

    meta = dict(
        chunks=chunks,
        diag_gid=diag_gid,
        last_of_window=last_of_window,
        nblocks=nblocks,
        tot_tiles=tot_tiles,
    )
    return idxw_all, s_all, meta


def _pool_matrices(batch):
    batch = np.asarray(batch, dtype=np.int64)
    cnt = np.bincount(batch, minlength=G).astype(np.float32)
    cnt = np.maximum(cnt, 1.0)
    g0 = np.zeros(NC, np.int64)
    s_pool = np.zeros((NC, NBLOCK128, 128, 128), np.float32)
    for c in range(NC):
        bl = batch[c * NLOC : (c + 1) * NLOC]
        g0[c] = bl[0]
        j = bl - g0[c]
        assert j.max() < 128, "graph span exceeds 128 on a core"
        val = (1.0 / cnt[bl]).astype(np.float32)
        sp = np.zeros((NPAD, 128), np.float32)
        sp[np.arange(NLOC), j] = val
        s_pool[c] = sp.reshape(NBLOCK128, 128, 128)
    return s_pool, g0


def _build(meta):
    chunks = meta["chunks"]
    diag_gid = meta["diag_gid"]
    last_of_window = meta["last_of_window"]
    nblocks = meta["nblocks"]
    tot_tiles = meta["tot_tiles"]

    tsb_max = max(ch["tiles"] for ch in chunks.values())
    pairs_max = max(len(ch["pairs"]) for ch in chunks.values())

    nc = bacc.Bacc()

    xT_p = nc.declare_dram_parameter("xT", [128, NPAD], F32, isOutput=False)
    W_p = nc.declare_dram_parameter("Wstack", [L, 128, 128], F32, isOutput=False)
    b_p = nc.declare_dram_parameter("bstack", [L, 128], F32, isOutput=False)
    idxw_p = nc.declare_dram_parameter(
        "idxw", [128, tot_tiles * 8], I16, isOutput=False
    )
    sflat_p = nc.declare_dram_parameter(
        "Sflat", [128, nblocks * 128], BF16, isOutput=False
    )
    spool_p = nc.declare_dram_parameter(
        "s_pool", [NBLOCK128, 128, 128], F32, isOutput=False
    )
    out_p = nc.declare_dram_parameter("out", [128, 128], F32, isOutput=True)

    shard = nc.dram_tensor("shard_bf16", [NPAD, F], BF16)
    tblk = [
        nc.dram_tensor(f"tblk{b}", [NBLKROWS, F], BF16, addr_space="Shared")
        for b in range(NBLK)
    ]

    with tile.TileContext(nc) as tc:
        with (
            tc.tile_pool(name="big", bufs=1) as big,
            tc.tile_pool(name="consts", bufs=1) as consts,
            tc.tile_pool(name="work", bufs=3) as work,
            tc.tile_pool(name="gath", bufs=2) as gath,
            tc.tile_pool(name="sld", bufs=2) as sldp,
            tc.tile_pool(name="dsl", bufs=2) as dslp,
            tc.tile_pool(name="ps", bufs=1, space="PSUM") as ps,
            tc.tile_pool(name="psagg", bufs=2, space="PSUM") as psagg,
        ):
            h = big.tile([128, NPAD], F32)
            sh_nm = big.tile([128, NBLOCK128, 128], BF16)  # shard, node-major
            Wt = consts.tile([128, L, 128], F32)
            bias = consts.tile([128, L], F32)
            ident = consts.tile([128, 128], BF16)
            make_identity(nc, ident[:])
            identf = consts.tile([128, 128], F32)
            make_identity(nc, identf[:])
            nc.sync.dma_start(out=h[:], in_=xT_p[:])
            nc.sync.dma_start(out=Wt[:], in_=W_p[:].rearrange("l a b -> a l b"))
            nc.sync.dma_start(out=bias[:], in_=b_p[:].rearrange("l f -> f l"))

            for layer in range(L):
                # ---- transform: h' = W^T @ h; shard row n = h'[:, n]
                for ch in range(NPAD // 512):
                    hp = ps.tile([128, 512], F32, name=f"hp_{layer}_{ch}", tag="hp")
                    nc.tensor.matmul(
                        out=hp[:],
                        lhsT=Wt[:, layer, :],
                        rhs=h[:, ch * 512 : (ch + 1) * 512],
                        start=True,
                        stop=True,
                    )
                    stg = work.tile(
                        [128, 512], BF16, name=f"stg_{layer}_{ch}", tag="stg"
                    )
                    nc.vector.tensor_copy(out=stg[:], in_=hp[:])
                    tst = work.tile(
                        [128, 4, 128], BF16, name=f"tst_{layer}_{ch}", tag="tst"
                    )
                    tpq = ps.tile(
                        [128, 4, 128], BF16, name=f"tpq_{layer}_{ch}", tag="tpq"
                    )
                    for j in range(4):
                        nc.tensor.transpose(
                            out=tpq[:, j, :],
                            in_=stg[:, j * 128 : (j + 1) * 128],
                            identity=ident[:],
                        )
                    nc.vector.tensor_copy(out=tst[:], in_=tpq[:])
                    r0 = ch * 512
                    nc.sync.dma_start(
                        out=shard[r0 : r0 + 512, :].rearrange(
                            "(b p) d -> p b d", p=128
                        ),
                        in_=tst[:],
                    )
                # node-major copy of the shard for the diag matmuls
                nc.sync.dma_start(
                    out=sh_nm[:],
                    in_=shard[:].rearrange("(b p) d -> p b d", p=128),
                )
                for b in range(NBLK):
                    nc.gpsimd.collective_compute(
                        "AllGather",
                        mybir.AluOpType.bypass,
                        replica_groups=[list(range(NC))],
                        ins=[shard[b * BLKLOC : (b + 1) * BLKLOC, :]],
                        outs=[tblk[b][:]],
                    )

                # ---- aggregation
                for sw in range(NSW):
                    w0, nw = _sw_windows(sw)
                    agg = psagg.tile(
                        [128, SWW * WIN],
                        F32,
                        name=f"agg_{layer}_{sw}",
                        tag="agg",
                        padded_shape=[128, SWW * WIN],
                    )
                    # diag (self-loop) matmuls open each window's chain
                    g0d = int(diag_gid[sw])
                    dsl = dslp.tile(
                        [128, nw * 128],
                        BF16,
                        name=f"dsl_{layer}_{sw}",
                        tag="dsl",
                        padded_shape=[128, SWW * 128],
                    )
                    nc.sync.dma_start(
                        out=dsl[:],
                        in_=sflat_p[:, g0d * 128 : (g0d + nw) * 128],
                    )
                    for wl in range(nw):
                        # ONE start per PSUM bank (4 windows of 512B each):
                        # start clears has_written for the WHOLE bank, so only
                        # the first matmul touching each bank may set it.
                        nc.tensor.matmul(
                            out=agg[:, wl * WIN : (wl + 1) * WIN],
                            lhsT=sh_nm[:, w0 + wl, :],
                            rhs=dsl[:, wl * 128 : (wl + 1) * 128],
                            start=(wl % 4 == 0),
                            stop=(w0 + wl) not in last_of_window,
                        )
                    for b in range(NBLK):
                        ch = chunks[(sw, b)]
                        tsb = ch["tiles"]
                        if tsb == 0:
                            continue
                        tbase = ch["tbase"]
                        idxs = gath.tile(
                            [128, tsb * 8],
                            I16,
                            name=f"idx_{layer}_{sw}_{b}",
                            tag="idx",
                            padded_shape=[128, tsb_max * 8],
                        )
                        nc.sync.dma_start(
                            out=idxs[:],
                            in_=idxw_p[:, tbase * 8 : (tbase + tsb) * 8],
                        )
                        msg = gath.tile(
                            [128, tsb, 128],
                            BF16,
                            name=f"msg_{layer}_{sw}_{b}",
                            tag="msg",
                            padded_shape=[128, tsb_max, 128],
                        )
                        for k0 in range(0, tsb, GCH):
                            kn = min(GCH, tsb - k0)
                            nc.gpsimd.dma_gather(
                                out_ap=msg[:, k0 : k0 + kn, :],
                                in_ap=tblk[b][:],
                                idxs_ap=idxs[:, k0 * 8 : (k0 + kn) * 8],
                                num_idxs=kn * 128,
                                num_idxs_reg=kn * 128,
                                elem_size=F,
                                single_packet=False,
                            )
                        npair = len(ch["pairs"])
                        g0p = ch["pairs"][0][2]
                        sld = sldp.tile(
                            [128, npair * 128],
                            BF16,
                            name=f"sld_{layer}_{sw}_{b}",
                            tag="sld",
                            padded_shape=[128, pairs_max * 128],
                        )
                        nc.sync.dma_start(
                            out=sld[:],
                            in_=sflat_p[:, g0p * 128 : (g0p + npair) * 128],
                        )
                        # merge consecutive-window pairs of the same msg
                        # tile into one wider matmul (bank-aligned, N<=512)
                        pairs = ch["pairs"]
                        runs = []  # [t, j0, wl0, ln]
                        for j, (t, wl, g) in enumerate(pairs):
                            if (
                                runs
                                and runs[-1][0] == t
                                and wl == runs[-1][2] + runs[-1][3]
                                and wl % 4 != 0
                            ):
                                runs[-1][3] += 1
                            else:
                                runs.append([t, j, wl, 1])
                        for t, j0, wl0, ln in runs:
                            stop = any(
                                last_of_window.get(w0 + wl) == (b, t, g)
                                for _, wl, g in pairs[j0 : j0 + ln]
                            )
                            nc.tensor.matmul(
                                out=agg[:, wl0 * WIN : (wl0 + ln) * WIN],
                                lhsT=msg[:, t, :],
                                rhs=sld[:, j0 * 128 : (j0 + ln) * 128],
                                start=False,
                                stop=stop,
                                skip_group_check=True,
                            )
                    for wl in range(nw):
                        nc.scalar.activation(
                            out=h[:, (w0 + wl) * WIN : (w0 + wl + 1) * WIN],
                            in_=agg[:, wl * WIN : (wl + 1) * WIN],
                            func=mybir.ActivationFunctionType.Relu,
                            bias=bias[:, layer : layer + 1],
                        )

            # ---- mean pool
            pool_ps = ps.tile([128, 512], F32, name="pool_ps", tag="hp")
            for blk in range(NBLOCK128):
                sp = work.tile([128, 128], F32, name=f"sp_{blk}", tag="sp")
                nc.sync.dma_start(out=sp[:], in_=spool_p[blk])
                tp2 = psagg.tile(
                    [128, SWW * WIN], F32, name=f"tp2_{blk}", tag="agg"
                )
                nc.tensor.transpose(
                    out=tp2[:, :128],
                    in_=h[:, blk * 128 : (blk + 1) * 128],
                    identity=identf[:],
                )
                hT = work.tile([128, 128], F32, name=f"hT_{blk}", tag="hT")
                nc.vector.tensor_copy(out=hT[:], in_=tp2[:, :128])
                nc.tensor.matmul(
                    out=pool_ps[:, :128],
                    lhsT=sp[:],
                    rhs=hT[:],
                    start=(blk == 0),
                    stop=(blk == NBLOCK128 - 1),
                )
            ores = work.tile([128, 128], F32)
            nc.vector.tensor_copy(out=ores[:], in_=pool_ps[:, :128])
            nc.sync.dma_start(out=out_p[:], in_=ores[:])

    nc.finalize()
    return nc


def kernel(x, edge_index, batch, W0, Wh, b):
    x = np.asarray(x, dtype=np.float32)
    W0 = np.asarray(W0, dtype=np.float32)
    Wh = np.asarray(Wh, dtype=np.float32)
    b = np.asarray(b, dtype=np.float32)

    idxw, s_all, meta = _preprocess(edge_index)
    s_pool, g0 = _pool_matrices(batch)

    Wstack = np.concatenate([W0[None], Wh], axis=0)

    in_maps = []
    for c in range(NC):
        xT = np.zeros((128, NPAD), np.float32)
        xT[:, :NLOC] = x[c * NLOC : (c + 1) * NLOC].T
        in_maps.append(
            {
                "xT": xT,
                "Wstack": Wstack,
                "bstack": b,
                "idxw": idxw[c],
                "Sflat": s_all[c],
                "s_pool": s_pool[c],
            }
        )

    nc = _build(meta)
    _install_profile_hook()
    import os

    trace = os.environ.get("GNN_TRACE", "0") == "1"
    res = run_bass_kernel_spmd(
        nc,
        in_maps,
        core_ids=list(range(NC)),
        trace=trace,
        tmpdir=os.environ.get("GNN_TRACE_DIR"),
    )
    if trace and res.exec_time_ns is not None:
        print(f"HW exec time: {res.exec_time_ns} ns")

    out = np.zeros((G, F), np.float32)
    for c in range(NC):
        oc = res.results[c]["out"]
        lo = int(g0[c])
        hi = min(G, lo + 128)
        out[lo:hi] += oc[: hi - lo]
    return out

